# revision 17
# baseline (speedup 1.0000x reference)
"""Trainium2 Bass kernel for nn_DUSPSA (SPSA on f(x)=x0^2+Q*x1^2, 1000 iters).

Math: each SPSA step is linear in x given the Rademacher product p = d0*d1:
    x' = M_k(p) x,  M_k = [[c1, -c2 p],[-c3 p, c4]]
so the 1000-step loop is a per-batch-element chain of 2x2 matrix products.
Host side re-encodes the delta bits: for each 8-step group the 8 sign bits
select one of 256 possible group transfer matrices from a per-group table
(the tables are pure functions of a, c, num_itr).  The device reduces the
128 group matrices per element with a log-depth product tree (fp16 until
the last few levels, then fp32) and applies the result to x0.

Data-parallel over the batch across 8 cores; per core 2048 elements laid
out as 128 partitions x 16 columns.

Note: consecutive dependent DVE ops in raw bass exhibit a read-after-write
pipeline hazard; every dependent pair below is separated by >=2 ops.
"""
import numpy as np

import concourse.bass as bass
import concourse.mybir as mybir
from concourse.bass_utils import run_bass_kernel_spmd

ALPHA, GAMMA, Q = 0.602, 0.101, 8.0
N_CORES = 8
BS = 16384
BPC = BS // N_CORES          # 2048 batch elements per core
P = 128                      # partitions
C = BPC // P                 # 16 batch columns per partition
NIT = 1000
NPAD = 1024
GS = 16                      # steps per host-encoded group
NG = NPAD // GS              # 128 group matrices per element
f32 = mybir.dt.float32
f16 = mybir.dt.float16
MUL = mybir.AluOpType.mult
ADD = mybir.AluOpType.add

_CACHED = {}


def _build_nc():
    import contextlib

    nc = bass.Bass("TRN2", target_bir_lowering=False, debug=False)
    gmat = nc.declare_dram_parameter("gmat", [P, NG * 4 * C], f16, isOutput=False)
    xin = nc.declare_dram_parameter("xin", [P, 2 * C], f32, isOutput=False)
    yout = nc.declare_dram_parameter("yout", [P, 2 * C], f32, isOutput=True)
    scratch = nc.dram_tensor("warm", (P, 2 * C), f32, kind="Internal")

    stack = contextlib.ExitStack()
    with stack:
        sb = lambda name, shape, dt=f32: stack.enter_context(nc.sbuf_tensor(name, shape, dt))
        gm = sb("gm", [P, NG * 4 * C], f16)
        lv = {
            32: sb("l32", [P, 32 * 4 * C], f16),
            16: sb("l16", [P, 16 * 4 * C], f16),
            8: sb("l8", [P, 8 * 4 * C], f16),
            4: sb("l4", [P, 4 * 4 * C], f16),
            2: sb("l2", [P, 2 * 4 * C], f16),
            1: sb("l1", [P, 1 * 4 * C], f32),
        }
        tmp16 = [sb(f"t16_{i}", [P, 16 * C], f16) for i in range(8)]
        tmp32 = [sb(f"t32_{i}", [P, 4 * C], f32) for i in range(8)]
        xt = sb("xt", [P, 2 * C])
        out_stage = sb("out_stage", [P, 2 * C])
        dummy = sb("spacer_t", [P, C], f16)
        dma_sems = [stack.enter_context(nc.semaphore(f"dma{i}")) for i in range(4)]
        dma_out = stack.enter_context(nc.semaphore("dmao"))
        done_sem = stack.enter_context(nc.semaphore("done"))
        block = stack.enter_context(nc.Block())

        @block.sync
        def _(sync):
            gsz = 4 * C
            sync.dma_start(out=xt[:], in_=xin[:]).then_inc(dma_sems[3], 16)
            for i, (lo, hi) in enumerate(((0, 16), (16, 32), (32, 64))):
                sync.dma_start(
                    out=gm[:, lo * gsz : hi * gsz], in_=gmat[:, lo * gsz : hi * gsz]
                ).then_inc(dma_sems[i], 16)
            sync.dma_start(out=scratch[:], in_=out_stage[:]).then_inc(dma_out, 16)
            sync.wait_ge(done_sem, 1)
            sync.dma_start(out=yout[:], in_=out_stage[:]).then_inc(dma_out, 16)

        @block.vector
        def _(vector):
            def g4(t, m):
                return t[:].rearrange("p (g e c) -> p g e c", g=m, e=4, c=C)

            def emit_level(src_t, m, dst_t, tmps, j0=0, j1=None):
                """Merge groups of src (m groups) into dst (m/2): dst[j] = src[2j+1] @ src[2j]."""
                if j1 is None:
                    j1 = m // 2
                nj = j1 - j0
                s = g4(src_t, m)
                d = g4(dst_t, m // 2)
                E = [s[:, 2 * j0 : 2 * j1 : 2, e, :] for e in range(4)]
                F = [s[:, 2 * j0 + 1 : 2 * j1 : 2, e, :] for e in range(4)]
                t = [tmps[i][:, 0 : nj * C].rearrange("p (j c) -> p j c", j=nj, c=C)
                     for i in range(8)]
                # products; order chosen so every dependent write->read pair
                # (within the level AND across level transitions) has >=2
                # other ops in between.  First two muls touch only entries
                # e1/e2, which the previous level's adds wrote first.
                vector.tensor_tensor(t[1], F[1], E[2], MUL)  # F01*E10
                vector.tensor_tensor(t[6], F[2], E[1], MUL)  # F10*E01
                vector.tensor_tensor(t[2], F[0], E[1], MUL)  # F00*E01
                vector.tensor_tensor(t[4], F[2], E[0], MUL)  # F10*E00
                vector.tensor_tensor(t[3], F[1], E[3], MUL)  # F01*E11
                vector.tensor_tensor(t[5], F[3], E[2], MUL)  # F11*E10
                vector.tensor_tensor(t[0], F[0], E[0], MUL)  # F00*E00
                vector.tensor_tensor(t[7], F[3], E[3], MUL)  # F11*E11
                # sums: e1, e2 first so the next level can start promptly
                vector.tensor_tensor(d[:, j0:j1, 1, :], t[2], t[3], ADD)  # O01
                vector.tensor_tensor(d[:, j0:j1, 2, :], t[4], t[5], ADD)  # O10
                vector.tensor_tensor(d[:, j0:j1, 0, :], t[0], t[1], ADD)  # O00
                vector.tensor_tensor(d[:, j0:j1, 3, :], t[6], t[7], ADD)  # O11

            def grs(t, m):
                return t[:].rearrange("p (g r s c) -> p g r s c", g=m, r=2, s=2, c=C)

            # level A: 64 -> 32, in 3 chunks to overlap the gmat DMA
            vector.wait_ge(dma_sems[0], 16)
            emit_level(gm, NG, lv[32], tmp16, 0, 8)
            vector.wait_ge(dma_sems[1], 16)
            emit_level(gm, NG, lv[32], tmp16, 8, 16)
            vector.wait_ge(dma_sems[2], 16)
            emit_level(gm, NG, lv[32], tmp16, 16, 32)
            emit_level(lv[32], 32, lv[16], tmp16)
            emit_level(lv[16], 16, lv[8], tmp16)
            emit_level(lv[8], 8, lv[4], tmp16)
            emit_level(lv[4], 4, lv[2], tmp16)
            # fused final merge + apply: y = G_hi @ (G_lo @ x)
            vector.wait_ge(dma_sems[3], 16)
            l2 = grs(lv[2], 2)
            xv = xt[:].rearrange("p (k c) -> p k c", k=2, c=C)
            ta = tmp32[0][:, 0 : 4 * C].rearrange("p (r k c) -> p r k c", r=2, k=2, c=C)
            tb = tmp32[1][:, 0 : 4 * C].rearrange("p (r k c) -> p r k c", r=2, k=2, c=C)
            y1 = tmp32[2][:, 0 : 2 * C].rearrange("p (k c) -> p k c", k=2, c=C)
            ov = out_stage[:].rearrange("p (r c) -> p r c", r=2, c=C)
            xb = xv.unsqueeze(1).broadcast_to((P, 2, 2, C))
            vector.tensor_copy(dummy[:], gm[:, 0:C])  # spacer
            vector.tensor_copy(dummy[:], gm[:, 0:C])  # spacer
            vector.tensor_tensor(ta, l2[:, 0], xb, MUL)
            vector.tensor_copy(dummy[:], gm[:, 0:C])  # spacer
            vector.tensor_copy(dummy[:], gm[:, 0:C])  # spacer
            vector.tensor_tensor(y1, ta[:, :, 0, :], ta[:, :, 1, :], ADD)
            vector.tensor_copy(dummy[:], gm[:, 0:C])  # spacer
            vector.tensor_copy(dummy[:], gm[:, 0:C])  # spacer
            vector.tensor_tensor(tb, l2[:, 1], y1.unsqueeze(1).broadcast_to((P, 2, 2, C)), MUL)
            vector.tensor_copy(dummy[:], gm[:, 0:C])  # spacer
            vector.tensor_copy(dummy[:], gm[:, 0:C])  # spacer
            vector.tensor_tensor(ov, tb[:, :, 0, :], tb[:, :, 1, :], ADD).then_inc(done_sem, 1)

    return nc


def _step_consts(a, n):
    A = int(np.floor(0.1 * n))
    k = np.arange(1, NPAD + 1, dtype=np.float64)
    ak = np.where(k <= n, float(a[0]) / (k + 1.0 + A) ** ALPHA, 0.0)
    c1 = 1.0 - 2.0 * ak
    c2 = 2.0 * ak * Q
    c3 = 2.0 * ak
    c4 = 1.0 - 2.0 * ak * Q
    return c1, c2, c3, c4


def _build_lut(a, n):
    """T[g, m, 2, 2]: product of the 8 step matrices of group g, signs from m's bits."""
    c1, c2, c3, c4 = _step_consts(a, n)
    pm = np.array([1.0, -1.0])  # bit 0 -> p=+1, bit 1 -> p=-1
    T = np.empty((NPAD, 2, 2, 2))
    T[:, :, 0, 0] = c1[:, None]
    T[:, :, 0, 1] = -c2[:, None] * pm[None, :]
    T[:, :, 1, 0] = -c3[:, None] * pm[None, :]
    T[:, :, 1, 1] = c4[:, None]
    while T.shape[0] > NG:
        nb = T.shape[1]
        Tn = np.matmul(T[1::2][:, None], T[0::2][:, :, None])  # (g, m_lo, m_hi, 2, 2)
        Tn = np.transpose(Tn, (0, 2, 1, 3, 4))                 # (g, m_hi, m_lo, 2, 2)
        T = np.ascontiguousarray(Tn).reshape(T.shape[0] // 2, nb * nb, 2, 2)
    return T  # (NG, 2**GS, 2, 2) float64


def _prep_in_maps(X0, a, c, delta_bits, n):
    T = _build_lut(a, n).astype(np.float16)
    xb = (delta_bits[..., 0] ^ delta_bits[..., 1]).astype(np.int64)  # (n, BS)
    xb_pad = np.zeros((NPAD, BS), np.int64)
    xb_pad[:n] = xb
    idx = (xb_pad.reshape(NG, GS, BS) << np.arange(GS)[None, :, None]).sum(1)
    entries = T[np.arange(NG)[:, None], idx]  # (NG, BS, 2, 2) f16
    x = X0.astype(np.float64) * 20.0 - 10.0   # (BS, 2)
    in_maps = []
    for ci in range(N_CORES):
        sl = slice(ci * BPC, (ci + 1) * BPC)
        e = entries[:, sl].reshape(NG, P, C, 2, 2)
        g = np.ascontiguousarray(np.transpose(e, (1, 0, 3, 4, 2))).reshape(P, NG * 4 * C)
        xc = np.ascontiguousarray(
            x[sl].reshape(P, C, 2).transpose(0, 2, 1).astype(np.float32)
        ).reshape(P, 2 * C)
        in_maps.append({"gmat": g, "xin": xc})
    return in_maps


def _gather(results):
    out = np.empty((BS, 2), np.float32)
    for ci in range(N_CORES):
        y = results[ci]["yout"]
        sl = slice(ci * BPC, (ci + 1) * BPC)
        out[sl, 0] = y[:, 0:C].reshape(BPC)
        out[sl, 1] = y[:, C : 2 * C].reshape(BPC)
    return out


def kernel(X0, a, c, delta_bits, num_itr, **run_kwargs):
    X0 = np.ascontiguousarray(np.asarray(X0, np.float32))
    a = np.asarray(a, np.float32)
    c = np.asarray(c, np.float32)
    delta_bits = np.ascontiguousarray(np.asarray(delta_bits, np.int32))
    n = int(num_itr)
    assert X0.shape == (BS, 2) and delta_bits.shape == (n, BS, 2) and n == NIT

    if "nc" not in _CACHED:
        _CACHED["nc"] = _build_nc()
    nc = _CACHED["nc"]

    in_maps = _prep_in_maps(X0, a, c, delta_bits, n)
    res = run_bass_kernel_spmd(nc, in_maps, core_ids=list(range(N_CORES)), **run_kwargs)
    out = _gather(res.results)
    if run_kwargs:
        return out, res
    return out


if __name__ == "__main__":
    rng = np.random.default_rng(0)
    X0 = rng.random((BS, 2), dtype=np.float32)
    a = np.full((NIT,), 0.01, np.float32)
    c = np.full((NIT,), 0.01, np.float32)
    db = rng.integers(0, 2, size=(NIT, BS, 2), dtype=np.int32)
    out = kernel(X0=X0, a=a, c=c, delta_bits=db, num_itr=NIT)
    print("kernel ran, out:", out.shape, out.dtype, float(np.abs(out).max()))


# revision 18
# speedup vs baseline: 1.0328x; 1.0328x over previous
"""Trainium2 Bass kernel for nn_DUSPSA (SPSA on f(x)=x0^2+Q*x1^2, 1000 iters).

Math: each SPSA step is linear in x given the Rademacher product p = d0*d1:
    x' = M_k(p) x,  M_k = [[c1, -c2 p],[-c3 p, c4]]
so the 1000-step loop is a per-batch-element chain of 2x2 matrix products.
Host side re-encodes the delta bits: for each 8-step group the 8 sign bits
select one of 256 possible group transfer matrices from a per-group table
(the tables are pure functions of a, c, num_itr).  The device reduces the
128 group matrices per element with a log-depth product tree (fp16 until
the last few levels, then fp32) and applies the result to x0.

Data-parallel over the batch across 8 cores; per core 2048 elements laid
out as 128 partitions x 16 columns.

Note: consecutive dependent DVE ops in raw bass exhibit a read-after-write
pipeline hazard; every dependent pair below is separated by >=2 ops.
"""
import numpy as np

import concourse.bass as bass
import concourse.mybir as mybir
from concourse.bass_utils import run_bass_kernel_spmd

ALPHA, GAMMA, Q = 0.602, 0.101, 8.0
N_CORES = 8
BS = 16384
BPC = BS // N_CORES          # 2048 batch elements per core
P = 128                      # partitions
C = BPC // P                 # 16 batch columns per partition
NIT = 1000
NPAD = 1024
GS = 16                      # steps per host-encoded group
NG = NPAD // GS              # 128 group matrices per element
f32 = mybir.dt.float32
f16 = mybir.dt.float16
MUL = mybir.AluOpType.mult
ADD = mybir.AluOpType.add

_CACHED = {}


def _build_nc():
    import contextlib

    nc = bass.Bass("TRN2", target_bir_lowering=False, debug=False)
    gmat = nc.declare_dram_parameter("gmat", [P, NG * 4 * C], f16, isOutput=False)
    xin = nc.declare_dram_parameter("xin", [P, 2 * C], f32, isOutput=False)
    yout = nc.declare_dram_parameter("yout", [P, 2 * C], f32, isOutput=True)
    scratch = nc.dram_tensor("warm", (P, 2 * C), f32, kind="Internal")

    stack = contextlib.ExitStack()
    with stack:
        sb = lambda name, shape, dt=f32: stack.enter_context(nc.sbuf_tensor(name, shape, dt))
        gm = sb("gm", [P, NG * 4 * C], f16)
        lv = {
            32: sb("l32", [P, 32 * 4 * C], f16),
            16: sb("l16", [P, 16 * 4 * C], f16),
            8: sb("l8", [P, 8 * 4 * C], f16),
            4: sb("l4", [P, 4 * 4 * C], f16),
            2: sb("l2", [P, 2 * 4 * C], f16),
            1: sb("l1", [P, 1 * 4 * C], f32),
        }
        tmp16 = [sb(f"t16_{i}", [P, 16 * C], f16) for i in range(8)]
        tmp32 = [sb(f"t32_{i}", [P, 4 * C], f32) for i in range(8)]
        xt = sb("xt", [P, 2 * C])
        out_stage = sb("out_stage", [P, 2 * C])
        dummy = sb("spacer_t", [P, C], f16)
        dma_sems = [stack.enter_context(nc.semaphore(f"dma{i}")) for i in range(4)]
        dma_out = stack.enter_context(nc.semaphore("dmao"))
        done_sem = stack.enter_context(nc.semaphore("done"))
        block = stack.enter_context(nc.Block())

        @block.sync
        def _(sync):
            gsz = 4 * C
            for i, (lo, hi) in enumerate(((0, 16), (16, 32), (32, 64))):
                sync.dma_start(
                    out=gm[:, lo * gsz : hi * gsz], in_=gmat[:, lo * gsz : hi * gsz]
                ).then_inc(dma_sems[i], 16)
            sync.dma_start(out=xt[:], in_=xin[:]).then_inc(dma_sems[3], 16)
            sync.wait_ge(done_sem, 1)
            sync.dma_start(out=yout[:], in_=out_stage[:]).then_inc(dma_out, 16)

        @block.vector
        def _(vector):
            def g4(t, m):
                return t[:].rearrange("p (g e c) -> p g e c", g=m, e=4, c=C)

            def emit_level(src_t, m, dst_t, tmps, j0=0, j1=None):
                """Merge groups of src (m groups) into dst (m/2): dst[j] = src[2j+1] @ src[2j]."""
                if j1 is None:
                    j1 = m // 2
                nj = j1 - j0
                s = g4(src_t, m)
                d = g4(dst_t, m // 2)
                E = [s[:, 2 * j0 : 2 * j1 : 2, e, :] for e in range(4)]
                F = [s[:, 2 * j0 + 1 : 2 * j1 : 2, e, :] for e in range(4)]
                t = [tmps[i][:, 0 : nj * C].rearrange("p (j c) -> p j c", j=nj, c=C)
                     for i in range(8)]
                # products; order chosen so every dependent write->read pair
                # (within the level AND across level transitions) has >=2
                # other ops in between.  First two muls touch only entries
                # e1/e2, which the previous level's adds wrote first.
                vector.tensor_tensor(t[1], F[1], E[2], MUL)  # F01*E10
                vector.tensor_tensor(t[6], F[2], E[1], MUL)  # F10*E01
                vector.tensor_tensor(t[2], F[0], E[1], MUL)  # F00*E01
                vector.tensor_tensor(t[4], F[2], E[0], MUL)  # F10*E00
                vector.tensor_tensor(t[3], F[1], E[3], MUL)  # F01*E11
                vector.tensor_tensor(t[5], F[3], E[2], MUL)  # F11*E10
                vector.tensor_tensor(t[0], F[0], E[0], MUL)  # F00*E00
                vector.tensor_tensor(t[7], F[3], E[3], MUL)  # F11*E11
                # sums: e1, e2 first so the next level can start promptly
                vector.tensor_tensor(d[:, j0:j1, 1, :], t[2], t[3], ADD)  # O01
                vector.tensor_tensor(d[:, j0:j1, 2, :], t[4], t[5], ADD)  # O10
                vector.tensor_tensor(d[:, j0:j1, 0, :], t[0], t[1], ADD)  # O00
                vector.tensor_tensor(d[:, j0:j1, 3, :], t[6], t[7], ADD)  # O11

            def grs(t, m):
                return t[:].rearrange("p (g r s c) -> p g r s c", g=m, r=2, s=2, c=C)

            # level A: 64 -> 32, in 3 chunks to overlap the gmat DMA
            vector.wait_ge(dma_sems[0], 16)
            emit_level(gm, NG, lv[32], tmp16, 0, 8)
            vector.wait_ge(dma_sems[1], 16)
            emit_level(gm, NG, lv[32], tmp16, 8, 16)
            vector.wait_ge(dma_sems[2], 16)
            emit_level(gm, NG, lv[32], tmp16, 16, 32)
            emit_level(lv[32], 32, lv[16], tmp16)
            emit_level(lv[16], 16, lv[8], tmp16)
            emit_level(lv[8], 8, lv[4], tmp16)
            emit_level(lv[4], 4, lv[2], tmp16)
            # fused final merge + apply: y = G_hi @ (G_lo @ x)
            vector.wait_ge(dma_sems[3], 16)
            l2 = grs(lv[2], 2)
            xv = xt[:].rearrange("p (k c) -> p k c", k=2, c=C)
            ta = tmp32[0][:, 0 : 4 * C].rearrange("p (r k c) -> p r k c", r=2, k=2, c=C)
            tb = tmp32[1][:, 0 : 4 * C].rearrange("p (r k c) -> p r k c", r=2, k=2, c=C)
            y1 = tmp32[2][:, 0 : 2 * C].rearrange("p (k c) -> p k c", k=2, c=C)
            ov = out_stage[:].rearrange("p (r c) -> p r c", r=2, c=C)
            xb = xv.unsqueeze(1).broadcast_to((P, 2, 2, C))
            vector.tensor_copy(dummy[:], gm[:, 0:C])  # spacer
            vector.tensor_copy(dummy[:], gm[:, 0:C])  # spacer
            vector.tensor_tensor(ta, l2[:, 0], xb, MUL)
            vector.tensor_copy(dummy[:], gm[:, 0:C])  # spacer
            vector.tensor_copy(dummy[:], gm[:, 0:C])  # spacer
            vector.tensor_tensor(y1, ta[:, :, 0, :], ta[:, :, 1, :], ADD)
            vector.tensor_copy(dummy[:], gm[:, 0:C])  # spacer
            vector.tensor_copy(dummy[:], gm[:, 0:C])  # spacer
            vector.tensor_tensor(tb, l2[:, 1], y1.unsqueeze(1).broadcast_to((P, 2, 2, C)), MUL)
            vector.tensor_copy(dummy[:], gm[:, 0:C])  # spacer
            vector.tensor_copy(dummy[:], gm[:, 0:C])  # spacer
            vector.tensor_tensor(ov, tb[:, :, 0, :], tb[:, :, 1, :], ADD).then_inc(done_sem, 1)

    return nc


def _step_consts(a, n):
    A = int(np.floor(0.1 * n))
    k = np.arange(1, NPAD + 1, dtype=np.float64)
    ak = np.where(k <= n, float(a[0]) / (k + 1.0 + A) ** ALPHA, 0.0)
    c1 = 1.0 - 2.0 * ak
    c2 = 2.0 * ak * Q
    c3 = 2.0 * ak
    c4 = 1.0 - 2.0 * ak * Q
    return c1, c2, c3, c4


def _build_lut(a, n):
    """T[g, m, 2, 2]: product of the 8 step matrices of group g, signs from m's bits."""
    c1, c2, c3, c4 = _step_consts(a, n)
    pm = np.array([1.0, -1.0])  # bit 0 -> p=+1, bit 1 -> p=-1
    T = np.empty((NPAD, 2, 2, 2))
    T[:, :, 0, 0] = c1[:, None]
    T[:, :, 0, 1] = -c2[:, None] * pm[None, :]
    T[:, :, 1, 0] = -c3[:, None] * pm[None, :]
    T[:, :, 1, 1] = c4[:, None]
    while T.shape[0] > NG:
        nb = T.shape[1]
        Tn = np.matmul(T[1::2][:, None], T[0::2][:, :, None])  # (g, m_lo, m_hi, 2, 2)
        Tn = np.transpose(Tn, (0, 2, 1, 3, 4))                 # (g, m_hi, m_lo, 2, 2)
        T = np.ascontiguousarray(Tn).reshape(T.shape[0] // 2, nb * nb, 2, 2)
    return T  # (NG, 2**GS, 2, 2) float64


def _prep_in_maps(X0, a, c, delta_bits, n):
    T = _build_lut(a, n).astype(np.float16)
    xb = (delta_bits[..., 0] ^ delta_bits[..., 1]).astype(np.int64)  # (n, BS)
    xb_pad = np.zeros((NPAD, BS), np.int64)
    xb_pad[:n] = xb
    idx = (xb_pad.reshape(NG, GS, BS) << np.arange(GS)[None, :, None]).sum(1)
    entries = T[np.arange(NG)[:, None], idx]  # (NG, BS, 2, 2) f16
    x = X0.astype(np.float64) * 20.0 - 10.0   # (BS, 2)
    in_maps = []
    for ci in range(N_CORES):
        sl = slice(ci * BPC, (ci + 1) * BPC)
        e = entries[:, sl].reshape(NG, P, C, 2, 2)
        g = np.ascontiguousarray(np.transpose(e, (1, 0, 3, 4, 2))).reshape(P, NG * 4 * C)
        xc = np.ascontiguousarray(
            x[sl].reshape(P, C, 2).transpose(0, 2, 1).astype(np.float32)
        ).reshape(P, 2 * C)
        in_maps.append({"gmat": g, "xin": xc})
    return in_maps


def _gather(results):
    out = np.empty((BS, 2), np.float32)
    for ci in range(N_CORES):
        y = results[ci]["yout"]
        sl = slice(ci * BPC, (ci + 1) * BPC)
        out[sl, 0] = y[:, 0:C].reshape(BPC)
        out[sl, 1] = y[:, C : 2 * C].reshape(BPC)
    return out


def kernel(X0, a, c, delta_bits, num_itr, **run_kwargs):
    X0 = np.ascontiguousarray(np.asarray(X0, np.float32))
    a = np.asarray(a, np.float32)
    c = np.asarray(c, np.float32)
    delta_bits = np.ascontiguousarray(np.asarray(delta_bits, np.int32))
    n = int(num_itr)
    assert X0.shape == (BS, 2) and delta_bits.shape == (n, BS, 2) and n == NIT

    if "nc" not in _CACHED:
        _CACHED["nc"] = _build_nc()
    nc = _CACHED["nc"]

    in_maps = _prep_in_maps(X0, a, c, delta_bits, n)
    res = run_bass_kernel_spmd(nc, in_maps, core_ids=list(range(N_CORES)), **run_kwargs)
    out = _gather(res.results)
    if run_kwargs:
        return out, res
    return out


if __name__ == "__main__":
    rng = np.random.default_rng(0)
    X0 = rng.random((BS, 2), dtype=np.float32)
    a = np.full((NIT,), 0.01, np.float32)
    c = np.full((NIT,), 0.01, np.float32)
    db = rng.integers(0, 2, size=(NIT, BS, 2), dtype=np.int32)
    out = kernel(X0=X0, a=a, c=c, delta_bits=db, num_itr=NIT)
    print("kernel ran, out:", out.shape, out.dtype, float(np.abs(out).max()))


# revision 19
# speedup vs baseline: 1.1740x; 1.1367x over previous
"""Trainium2 Bass kernel for nn_DUSPSA (SPSA on f(x)=x0^2+Q*x1^2, 1000 iters).

Math: each SPSA step is linear in x given the Rademacher product p = d0*d1:
    x' = M_k(p) x,  M_k = [[c1, -c2 p],[-c3 p, c4]]
so the 1000-step loop is a per-batch-element chain of 2x2 matrix products.
Host side re-encodes the delta bits: for each 8-step group the 8 sign bits
select one of 256 possible group transfer matrices from a per-group table
(the tables are pure functions of a, c, num_itr).  The device reduces the
128 group matrices per element with a log-depth product tree (fp16 until
the last few levels, then fp32) and applies the result to x0.

Data-parallel over the batch across 8 cores; per core 2048 elements laid
out as 128 partitions x 16 columns.

Note: consecutive dependent DVE ops in raw bass exhibit a read-after-write
pipeline hazard; every dependent pair below is separated by >=2 ops.
"""
import numpy as np

import concourse.bass as bass
import concourse.mybir as mybir
from concourse.bass_utils import run_bass_kernel_spmd

ALPHA, GAMMA, Q = 0.602, 0.101, 8.0
N_CORES = 8
BS = 16384
BPC = BS // N_CORES          # 2048 batch elements per core
P = 128                      # partitions
C = BPC // P                 # 16 batch columns per partition
NIT = 1000
NPAD = 1024
GS = 16                      # steps per host-encoded group
NG = NPAD // GS              # 128 group matrices per element
f32 = mybir.dt.float32
f16 = mybir.dt.float16
MUL = mybir.AluOpType.mult
ADD = mybir.AluOpType.add

_CACHED = {}


def _build_nc():
    import contextlib

    nc = bass.Bass("TRN2", target_bir_lowering=False, debug=False)
    gmat = nc.declare_dram_parameter("gmat", [P, NG * 4 * C], f16, isOutput=False)
    xin = nc.declare_dram_parameter("xin", [P, 2 * C], f32, isOutput=False)
    yout = nc.declare_dram_parameter("yout", [P, 2 * C], f32, isOutput=True)

    stack = contextlib.ExitStack()
    with stack:
        sb = lambda name, shape, dt=f32: stack.enter_context(nc.sbuf_tensor(name, shape, dt))
        gm = sb("gm", [P, NG * 4 * C], f16)
        lv = {
            32: sb("l32", [P, 32 * 4 * C], f16),
            16: sb("l16", [P, 16 * 4 * C], f16),
            8: sb("l8", [P, 8 * 4 * C], f16),
            4: sb("l4", [P, 4 * 4 * C], f32),
            2: sb("l2", [P, 2 * 4 * C], f32),
            1: sb("l1", [P, 1 * 4 * C], f32),
        }
        tmp16 = [sb(f"t16_{i}", [P, 16 * C], f16) for i in range(8)]
        tmp32 = [sb(f"t32_{i}", [P, 4 * C], f32) for i in range(8)]
        xt = sb("xt", [P, 2 * C])
        out_stage = sb("out_stage", [P, 2 * C])
        dummy = sb("spacer_t", [P, C])
        dma_sems = [stack.enter_context(nc.semaphore(f"dma{i}")) for i in range(4)]
        dma_out = stack.enter_context(nc.semaphore("dmao"))
        done_sem = stack.enter_context(nc.semaphore("done"))
        block = stack.enter_context(nc.Block())

        @block.sync
        def _(sync):
            gsz = 4 * C
            for i, (lo, hi) in enumerate(((0, 16), (16, 32), (32, 64))):
                sync.dma_start(
                    out=gm[:, lo * gsz : hi * gsz], in_=gmat[:, lo * gsz : hi * gsz]
                ).then_inc(dma_sems[i], 16)
            sync.dma_start(out=xt[:], in_=xin[:]).then_inc(dma_sems[3], 16)
            sync.wait_ge(done_sem, 1)
            sync.dma_start(out=yout[:], in_=out_stage[:]).then_inc(dma_out, 16)

        @block.vector
        def _(vector):
            def g4(t, m):
                return t[:].rearrange("p (g e c) -> p g e c", g=m, e=4, c=C)

            def emit_level(src_t, m, dst_t, tmps, j0=0, j1=None):
                """Merge groups of src (m groups) into dst (m/2): dst[j] = src[2j+1] @ src[2j]."""
                if j1 is None:
                    j1 = m // 2
                nj = j1 - j0
                s = g4(src_t, m)
                d = g4(dst_t, m // 2)
                E = [s[:, 2 * j0 : 2 * j1 : 2, e, :] for e in range(4)]
                F = [s[:, 2 * j0 + 1 : 2 * j1 : 2, e, :] for e in range(4)]
                t = [tmps[i][:, 0 : nj * C].rearrange("p (j c) -> p j c", j=nj, c=C)
                     for i in range(8)]
                # products; order chosen so every dependent write->read pair
                # (within the level AND across level transitions) has >=2
                # other ops in between.  First two muls touch only entries
                # e1/e2, which the previous level's adds wrote first.
                vector.tensor_tensor(t[1], F[1], E[2], MUL)  # F01*E10
                vector.tensor_tensor(t[6], F[2], E[1], MUL)  # F10*E01
                vector.tensor_tensor(t[2], F[0], E[1], MUL)  # F00*E01
                vector.tensor_tensor(t[4], F[2], E[0], MUL)  # F10*E00
                vector.tensor_tensor(t[3], F[1], E[3], MUL)  # F01*E11
                vector.tensor_tensor(t[5], F[3], E[2], MUL)  # F11*E10
                vector.tensor_tensor(t[0], F[0], E[0], MUL)  # F00*E00
                vector.tensor_tensor(t[7], F[3], E[3], MUL)  # F11*E11
                # sums: e1, e2 first so the next level can start promptly
                vector.tensor_tensor(d[:, j0:j1, 1, :], t[2], t[3], ADD)  # O01
                vector.tensor_tensor(d[:, j0:j1, 2, :], t[4], t[5], ADD)  # O10
                vector.tensor_tensor(d[:, j0:j1, 0, :], t[0], t[1], ADD)  # O00
                vector.tensor_tensor(d[:, j0:j1, 3, :], t[6], t[7], ADD)  # O11

            def grs(t, m):
                return t[:].rearrange("p (g r s c) -> p g r s c", g=m, r=2, s=2, c=C)

            # level A: 64 -> 32, in 3 chunks to overlap the gmat DMA
            vector.wait_ge(dma_sems[0], 16)
            emit_level(gm, NG, lv[32], tmp16, 0, 8)
            vector.wait_ge(dma_sems[1], 16)
            emit_level(gm, NG, lv[32], tmp16, 8, 16)
            vector.wait_ge(dma_sems[2], 16)
            emit_level(gm, NG, lv[32], tmp16, 16, 32)
            emit_level(lv[32], 32, lv[16], tmp16)
            emit_level(lv[16], 16, lv[8], tmp16)
            emit_level(lv[8], 8, lv[4], tmp32)
            emit_level(lv[4], 4, lv[2], tmp32)
            # fused final merge + apply: y = G_hi @ (G_lo @ x)
            vector.wait_ge(dma_sems[3], 16)
            l2 = grs(lv[2], 2)
            xv = xt[:].rearrange("p (k c) -> p k c", k=2, c=C)
            ta = tmp32[0][:, 0 : 4 * C].rearrange("p (r k c) -> p r k c", r=2, k=2, c=C)
            tb = tmp32[1][:, 0 : 4 * C].rearrange("p (r k c) -> p r k c", r=2, k=2, c=C)
            y1 = tmp32[2][:, 0 : 2 * C].rearrange("p (k c) -> p k c", k=2, c=C)
            ov = out_stage[:].rearrange("p (r c) -> p r c", r=2, c=C)
            xb = xv.unsqueeze(1).broadcast_to((P, 2, 2, C))
            vector.tensor_copy(dummy[:], gm[:, 0:C])  # spacer
            vector.tensor_copy(dummy[:], gm[:, 0:C])  # spacer
            vector.tensor_tensor(ta, l2[:, 0], xb, MUL)
            vector.tensor_copy(dummy[:], gm[:, 0:C])  # spacer
            vector.tensor_copy(dummy[:], gm[:, 0:C])  # spacer
            vector.tensor_tensor(y1, ta[:, :, 0, :], ta[:, :, 1, :], ADD)
            vector.tensor_copy(dummy[:], gm[:, 0:C])  # spacer
            vector.tensor_copy(dummy[:], gm[:, 0:C])  # spacer
            vector.tensor_tensor(tb, l2[:, 1], y1.unsqueeze(1).broadcast_to((P, 2, 2, C)), MUL)
            vector.tensor_copy(dummy[:], gm[:, 0:C])  # spacer
            vector.tensor_copy(dummy[:], gm[:, 0:C])  # spacer
            vector.tensor_tensor(ov, tb[:, :, 0, :], tb[:, :, 1, :], ADD).then_inc(done_sem, 1)

    return nc


def _step_consts(a, n):
    A = int(np.floor(0.1 * n))
    k = np.arange(1, NPAD + 1, dtype=np.float64)
    ak = np.where(k <= n, float(a[0]) / (k + 1.0 + A) ** ALPHA, 0.0)
    c1 = 1.0 - 2.0 * ak
    c2 = 2.0 * ak * Q
    c3 = 2.0 * ak
    c4 = 1.0 - 2.0 * ak * Q
    return c1, c2, c3, c4


def _build_lut(a, n):
    """T[g, m, 2, 2]: product of the 8 step matrices of group g, signs from m's bits."""
    c1, c2, c3, c4 = _step_consts(a, n)
    pm = np.array([1.0, -1.0])  # bit 0 -> p=+1, bit 1 -> p=-1
    T = np.empty((NPAD, 2, 2, 2))
    T[:, :, 0, 0] = c1[:, None]
    T[:, :, 0, 1] = -c2[:, None] * pm[None, :]
    T[:, :, 1, 0] = -c3[:, None] * pm[None, :]
    T[:, :, 1, 1] = c4[:, None]
    while T.shape[0] > NG:
        nb = T.shape[1]
        Tn = np.matmul(T[1::2][:, None], T[0::2][:, :, None])  # (g, m_lo, m_hi, 2, 2)
        Tn = np.transpose(Tn, (0, 2, 1, 3, 4))                 # (g, m_hi, m_lo, 2, 2)
        T = np.ascontiguousarray(Tn).reshape(T.shape[0] // 2, nb * nb, 2, 2)
    return T  # (NG, 2**GS, 2, 2) float64


def _prep_in_maps(X0, a, c, delta_bits, n):
    T = _build_lut(a, n).astype(np.float16)
    xb = (delta_bits[..., 0] ^ delta_bits[..., 1]).astype(np.int64)  # (n, BS)
    xb_pad = np.zeros((NPAD, BS), np.int64)
    xb_pad[:n] = xb
    idx = (xb_pad.reshape(NG, GS, BS) << np.arange(GS)[None, :, None]).sum(1)
    entries = T[np.arange(NG)[:, None], idx]  # (NG, BS, 2, 2) f16
    x = X0.astype(np.float64) * 20.0 - 10.0   # (BS, 2)
    in_maps = []
    for ci in range(N_CORES):
        sl = slice(ci * BPC, (ci + 1) * BPC)
        e = entries[:, sl].reshape(NG, P, C, 2, 2)
        g = np.ascontiguousarray(np.transpose(e, (1, 0, 3, 4, 2))).reshape(P, NG * 4 * C)
        xc = np.ascontiguousarray(
            x[sl].reshape(P, C, 2).transpose(0, 2, 1).astype(np.float32)
        ).reshape(P, 2 * C)
        in_maps.append({"gmat": g, "xin": xc})
    return in_maps


def _gather(results):
    out = np.empty((BS, 2), np.float32)
    for ci in range(N_CORES):
        y = results[ci]["yout"]
        sl = slice(ci * BPC, (ci + 1) * BPC)
        out[sl, 0] = y[:, 0:C].reshape(BPC)
        out[sl, 1] = y[:, C : 2 * C].reshape(BPC)
    return out


def kernel(X0, a, c, delta_bits, num_itr, **run_kwargs):
    X0 = np.ascontiguousarray(np.asarray(X0, np.float32))
    a = np.asarray(a, np.float32)
    c = np.asarray(c, np.float32)
    delta_bits = np.ascontiguousarray(np.asarray(delta_bits, np.int32))
    n = int(num_itr)
    assert X0.shape == (BS, 2) and delta_bits.shape == (n, BS, 2) and n == NIT

    if "nc" not in _CACHED:
        _CACHED["nc"] = _build_nc()
    nc = _CACHED["nc"]

    in_maps = _prep_in_maps(X0, a, c, delta_bits, n)
    res = run_bass_kernel_spmd(nc, in_maps, core_ids=list(range(N_CORES)), **run_kwargs)
    out = _gather(res.results)
    if run_kwargs:
        return out, res
    return out


if __name__ == "__main__":
    rng = np.random.default_rng(0)
    X0 = rng.random((BS, 2), dtype=np.float32)
    a = np.full((NIT,), 0.01, np.float32)
    c = np.full((NIT,), 0.01, np.float32)
    db = rng.integers(0, 2, size=(NIT, BS, 2), dtype=np.int32)
    out = kernel(X0=X0, a=a, c=c, delta_bits=db, num_itr=NIT)
    print("kernel ran, out:", out.shape, out.dtype, float(np.abs(out).max()))
